# revision 24
# baseline (speedup 1.0000x reference)
"""GATv2 multi-head attention kernel for Trainium2 (8 NeuronCores).

Problem: nn_GATv2MHA  (b=4, n=512, input_dim=128, 8 heads x head_dim 16)
  g_l = einsum('bni,hid->hbnd', h, Wl); g_r likewise
  e = leaky_relu(g_l[:,:,:,None,:] + g_r[:,:,None,:,:], 0.2)
  scores = einsum('hbijd,hd->hbij', e, Wak);  attn = softmax(scores, -1)
  out = relu(einsum('hbij,hbjd->bihd', attn, g_r)).reshape(b, n, -1)

Sharding: data-parallel over (batch, token-half): core c handles batch c//2,
token rows [(c%2)*256, (c%2)*256+256).  No collectives.

Math trick: LeakyRelu(x) = 0.2*x + 0.8*relu(x), and the 0.2*u_i row-constant
cancels in softmax, so
  scores'[h,i,j] = 0.8 * sum_d a_hd relu(gl_hid + gr_hjd) + 0.2 * v_hj
with v_hj = sum_d a_hd gr_hjd.  relu(gl+gr) is ONE tensor_scalar(add,max) op
per row i on DVE (or activation(Relu, bias) on ACT), and the a-contraction is
done on TensorE with per-i block-diagonal weights WS_q accumulating into one
PSUM tile of 16 rows x 8 heads = 128 partitions; the Wv matmul (0.2*v_j term)
runs FIRST in each accumulation group (start=True) since it has no X dep.

v2 structure vs v1:
  - P is bf16 and exp() is taken WITHOUT max-subtraction (bf16 range is ample
    for exp(|s|<~50)); kills the per-block DVE reduce_max + its serial dep.
  - P -> PT transpose and grT -> gr_cat both use the XBAR DMA transpose
    (dma_start_transpose) on the otherwise-idle DMA engines, removing 4 PE
    transposes + 4 engine copies per block (and 2 PSUM banks).
  - Softmax denominator from the exp activation accumulator (accum_out).
  - Tail steps of block k are emitted at staggered q-slots of block k+1 so the
    out-proj matmuls never head-of-line-block the PE queue while the DMA
    transpose is in flight.
  - h / hTg / Wl / Wr ship as f16 (half the input DMA bytes; projections
    run at 1 cycle/row instead of fp32's 4).
  - A short burst of dummy matmuls on a memset tile warms the PE p-state
    (low->mid->full over ~3us busy) during the input-DMA wait.
  - Output: per block, the normalized [128,128] tile is masked to its head
    block-diagonal (DVE multiply) and compacted to 16 rows by a gather
    matmul (lhsT sums over heads), so the final output ships as a few
    contiguous 256B-run DMAs instead of many fragmented diagonal DMAs.
"""

import functools
import os

import numpy as np
import ml_dtypes

N_HEADS = 8
INPUT_DIM = 128
HEAD_DIM = 16
B = 4
N = 512
N_CORES = 8
HALF = N // 2          # token rows per core
BLK = 16               # i-rows per block (x 8 heads = 128 partitions)
NBLK = HALF // BLK     # 16 blocks per core

# ---- tunables (overridable via env for experiments) ----------------------
ACT_QS = int(os.environ.get("GAT_ACT_QS", "3"))  # X-ops on ScalarE per block
XBUFS = int(os.environ.get("GAT_XBUFS", "10"))
SBUFS = int(os.environ.get("GAT_SBUFS", "5"))    # PSUM score banks
PBUFS = int(os.environ.get("GAT_PBUFS", "2"))
PTBUFS = int(os.environ.get("GAT_PTBUFS", "3"))
DMA_GRP = int(os.environ.get("GAT_DMA_GRP", "4"))  # blocks per diag-DMA group
# q-slots (in the NEXT block) at which the previous block's tail steps run
Q_EXP = int(os.environ.get("GAT_Q_EXP", "0"))
Q_TR = int(os.environ.get("GAT_Q_TR", "1"))
Q_OP = int(os.environ.get("GAT_Q_OP", "11"))
Q_ZI = int(os.environ.get("GAT_Q_ZI", "13"))
Q_NORM = int(os.environ.get("GAT_Q_NORM", "16"))
# must be > Q_NORM: group g's diag DMAs read the normalize output of the
# group's last block, which is emitted at Q_NORM of the following block
Q_DMA = int(os.environ.get("GAT_Q_DMA", "16"))


def build_program():
    """Build + compile the (identical-across-cores) Bass program."""
    import concourse.bass as bass
    import concourse.mybir as mybir
    import concourse.tile as tile
    from concourse import bacc

    f32 = mybir.dt.float32
    f16 = mybir.dt.float16
    bf16 = mybir.dt.bfloat16

    nc = bacc.Bacc("TRN2", target_bir_lowering=False, debug=False)

    hT = nc.dram_tensor("hT", (128, N), f16, kind="ExternalInput").ap()
    hTg = nc.dram_tensor("hTg", (128, HALF), f16, kind="ExternalInput").ap()
    WlT = nc.dram_tensor("WlT", (128, 128), f16, kind="ExternalInput").ap()
    WrT = nc.dram_tensor("WrT", (128, 128), f16, kind="ExternalInput").ap()
    WS = nc.dram_tensor("WS", (128, BLK, 128), f16, kind="ExternalInput").ap()
    Wv = nc.dram_tensor("Wv", (128, 128), f16, kind="ExternalInput").ap()
    G16 = nc.dram_tensor("G16", (128, 16), f16, kind="ExternalInput").ap()
    HM = nc.dram_tensor("HM", (128, 128), f16, kind="ExternalInput").ap()
    od = nc.dram_tensor("out_d", (HALF, 128), bf16, kind="ExternalOutput").ap()

    ADD = mybir.AluOpType.add
    MAX = mybir.AluOpType.max
    RELU = mybir.ActivationFunctionType.Relu
    EXP = mybir.ActivationFunctionType.Exp

    with tile.TileContext(nc) as tc:
        with (
            tc.tile_pool(name="singles", bufs=1) as singles,
            tc.tile_pool(name="xpool", bufs=XBUFS) as xpool,
            tc.tile_pool(name="ppool", bufs=PBUFS) as ppool,
            tc.tile_pool(name="ptpool", bufs=PTBUFS) as ptpool,
            tc.tile_pool(name="small", bufs=6) as small,
            tc.tile_pool(name="ps_s", bufs=SBUFS, space=bass.MemorySpace.PSUM) as ps_s,
            tc.tile_pool(name="ps_o", bufs=2, space=bass.MemorySpace.PSUM) as ps_o,
            tc.tile_pool(name="ps_g", bufs=1, space=bass.MemorySpace.PSUM) as ps_g,
        ):
            # ---- input DMAs: critical-path-ordered across the two HWDGE
            # queues.  scalar queue: weights for the projections + first WS
            # columns; sync queue: the big hT load.
            sb_hT = singles.tile([128, N], f16)
            nc.sync.dma_start(sb_hT[:, 0:256], hT[:, 0:256])
            sb_WrT = singles.tile([128, 128], f16)
            nc.scalar.dma_start(sb_WrT, WrT)
            nc.scalar.dma_start(sb_hT[:, 256:512], hT[:, 256:512])
            sb_WlT = singles.tile([128, 128], f16)
            nc.scalar.dma_start(sb_WlT, WlT)
            sb_hTg = singles.tile([128, HALF], f16)
            nc.sync.dma_start(sb_hTg, hTg)
            sb_WS = singles.tile([128, BLK, 128], f16)
            nc.sync.dma_start(sb_WS[:, 0:4], WS[:, 0:4])
            sb_Wv = singles.tile([128, 128], f16)
            nc.scalar.dma_start(sb_Wv, Wv)
            sb_G16 = singles.tile([128, 16], f16)
            nc.scalar.dma_start(sb_G16, G16)
            nc.sync.dma_start(sb_WS[:, 4:10], WS[:, 4:10])
            nc.scalar.dma_start(sb_WS[:, 10:16], WS[:, 10:16])

            # ---- PE clock warm-up: dummy matmuls on a memset tile run
            # during the otherwise-idle input-DMA wait, so the PE p-state
            # ramp (low->mid->full over ~3us of busy time) completes before
            # the real score matmuls arrive.
            warm = singles.tile([128, N], f16)
            nc.vector.memset(warm, 0.0)
            w_ps = ps_s.tile([128, N], f32, tag="S")
            NWARM = int(os.environ.get("GAT_NWARM", "4"))
            for wi in range(NWARM):
                nc.tensor.matmul(w_ps, lhsT=warm[:, 0:128], rhs=warm,
                                 start=(wi == 0), stop=(wi == NWARM - 1))

            # ---- prolog: projections ---------------------------------
            # grT[(h,d), j] for all 512 j (cast to f16); the copy out of PSUM
            # is split across DVE/ACT so X-ops can start sooner.
            r_ps = ps_s.tile([128, N], f32, tag="S")
            nc.tensor.matmul(r_ps, lhsT=sb_WrT, rhs=sb_hT,
                             start=True, stop=True)
            sb_grT = singles.tile([128, N], f16)
            nc.vector.tensor_copy(sb_grT[:, 0:256], r_ps[:, 0:256])
            nc.scalar.copy(sb_grT[:, 256:512], r_ps[:, 256:512])

            # glT[(h,d), i_local] (kept fp32: read as per-partition scalars)
            g_ps = ps_s.tile([128, HALF], f32, tag="S")
            nc.tensor.matmul(g_ps, lhsT=sb_WlT, rhs=sb_hTg,
                             start=True, stop=True)
            sb_glT = singles.tile([128, HALF], f32)
            nc.vector.tensor_copy(sb_glT[:, 0:128], g_ps[:, 0:128])
            nc.scalar.copy(sb_glT[:, 128:256], g_ps[:, 128:256])

            # gr_cat[j,(h,d)] via XBAR DMA transpose of grT (packed layout).
            # The j <-> (chunk, partition) mapping matches the P->PT transpose
            # below, so the out-proj contraction covers each j exactly once.
            sb_grcat = singles.tile([128, 4, 128], f16)
            nc.sync.dma_start_transpose(sb_grcat, sb_grT)

            # block-diagonal head mask: MASK[(h,i2),(h2,d)] = (h == h2)
            sb_mask = singles.tile([128, 128], f16)
            nc.scalar.dma_start(sb_mask, HM)

            # compact per-block outputs: stage_c[i2, blk, (h,d)] holds the
            # head-diagonal rows of each block, gathered across partitions by
            # a mask-multiply + 16-row gather matmul
            stage_c = singles.tile([16, NBLK, 128], bf16)

            # ---- tail steps for block blk, staggered into the next block --
            def make_tail(blk, S_ps):
                state = {}

                def t_exp():
                    P = ppool.tile([128, N], bf16, tag="P")
                    Z = small.tile([128, 1], f32, tag="Z", name=f"Z{blk}")
                    nc.scalar.activation(P, S_ps, EXP, accum_out=Z)
                    state["P"] = P
                    state["Z"] = Z

                def t_tr():
                    PT = ptpool.tile([128, 4, 128], bf16, tag="PT")
                    nc.sync.dma_start_transpose(PT, state["P"])
                    state["PT"] = PT

                def t_op():
                    O_ps = ps_o.tile([128, 128], f32, tag="O")
                    for cch in range(4):
                        nc.tensor.matmul(
                            O_ps,
                            lhsT=state["PT"][:, cch],
                            rhs=sb_grcat[:, cch],
                            start=(cch == 0),
                            stop=(cch == 3),
                        )
                    state["O"] = O_ps

                def t_zi():
                    Zi = small.tile([128, 1], f32, tag="Zi", name=f"Zi{blk}")
                    nc.vector.reciprocal(Zi, state["Z"])
                    state["Zi"] = Zi

                def t_norm():
                    Nn = ppool.tile([128, 128], bf16, tag="N")
                    nc.scalar.activation(
                        Nn, state["O"], RELU, bias=0.0, scale=state["Zi"],
                    )
                    state["N"] = Nn

                def t_mask():
                    Nm = ppool.tile([128, 128], bf16, tag="Nm")
                    nc.vector.tensor_tensor(
                        Nm, state["N"], sb_mask, mybir.AluOpType.mult
                    )
                    state["Nm"] = Nm

                def t_gath():
                    o16 = ps_g.tile([16, 128], f32, tag="G")
                    nc.tensor.matmul(o16, lhsT=sb_G16, rhs=state["Nm"],
                                     start=True, stop=True)
                    nc.vector.tensor_copy(stage_c[:, blk], o16)

                return [
                    (Q_EXP, t_exp),
                    (Q_TR, t_tr),
                    (Q_OP, t_op),
                    (Q_ZI, t_zi),
                    (Q_NORM, t_norm),
                    (Q_NORM + 1, t_mask),
                    (Q_NORM + 2, t_gath),
                ]

            # ship compact output blocks: out_d[(b0+blk)*16+i2, (h,d)]
            #   = stage_c[i2, b0+blk, (h,d)]  (contiguous 256B runs)
            def make_odma(b0, nblks):
                def run():
                    dst = bass.AP(
                        tensor=od.tensor,
                        offset=b0 * BLK * 128,
                        # dims iterate (i2, blk, hd) matching the src view
                        ap=[[128, BLK], [BLK * 128, nblks], [1, 128]],
                    )
                    nc.sync.dma_start(dst, stage_c[:, b0 : b0 + nblks])

                return run

            # ---- main loop: 16 blocks of 16 token-rows ---------------
            pending = []  # [(q_slot, fn)] for the current block
            for blk in range(NBLK):
                S_ps = ps_s.tile([128, N], f32, tag="S", name=f"S{blk}")
                # dep-free 0.2*v_j term opens the accumulation group
                nc.tensor.matmul(S_ps, lhsT=sb_Wv, rhs=sb_grT,
                                 start=True, stop=False)
                for q in range(BLK):
                    for qs, fn in pending:
                        if qs == q:
                            fn()
                    i = blk * BLK + q
                    X = xpool.tile([128, N], f16, tag="X")
                    gl_col = sb_glT[:, i : i + 1]
                    if q < BLK - ACT_QS:
                        nc.vector.tensor_scalar(X, sb_grT, gl_col, 0.0, ADD, MAX)
                    else:
                        nc.scalar.activation(X, sb_grT, RELU, bias=gl_col,
                                             scale=1.0)
                    nc.tensor.matmul(
                        S_ps,
                        lhsT=sb_WS[:, q],
                        rhs=X,
                        start=False,
                        stop=(q == BLK - 1),
                    )
                for qs, fn in pending:
                    if qs >= BLK:
                        fn()
                pending = make_tail(blk, S_ps)
                if blk in (4, 8, 12):
                    # stage_c rows for blocks blk-4..blk-1 are complete
                    pending.append((3, make_odma(blk - 4, 4)))
                elif blk == 15:
                    pending.append((3, make_odma(12, 2)))
            # flush the last tail, then the final two blocks' output DMA
            for _, fn in sorted(pending, key=lambda t: t[0]):
                fn()
            make_odma(14, 2)()

    nc.compile()
    return nc


@functools.lru_cache(maxsize=1)
def get_program():
    return build_program()


def host_prep(h, Wl, Wr, Wak):
    """Build per-core input maps (all numpy, no device work)."""
    h = np.asarray(h, dtype=np.float32)
    Wl = np.asarray(Wl, dtype=np.float32)
    Wr = np.asarray(Wr, dtype=np.float32)
    Wak = np.asarray(Wak, dtype=np.float32)

    hT_all = np.ascontiguousarray(h.transpose(0, 2, 1)).astype(np.float16)
    WlT = np.ascontiguousarray(
        Wl.transpose(1, 0, 2).reshape(INPUT_DIM, N_HEADS * HEAD_DIM)
    ).astype(np.float16)
    WrT = np.ascontiguousarray(
        Wr.transpose(1, 0, 2).reshape(INPUT_DIM, N_HEADS * HEAD_DIM)
    ).astype(np.float16)

    # WS[q][(h,d), (h2,i2)] = 0.8 * Wak[h,d] * (h==h2) * (i2==q)
    WS = np.zeros((128, BLK, 128), dtype=np.float32)
    for hh in range(N_HEADS):
        for q in range(BLK):
            WS[hh * 16 : hh * 16 + 16, q, hh * 16 + q] = 0.8 * Wak[hh]
    # Wv[(h,d), (h2,i2)] = 0.2 * Wak[h,d] * (h==h2)   (all i2)
    Wv = np.zeros((128, 128), dtype=np.float32)
    for hh in range(N_HEADS):
        Wv[hh * 16 : hh * 16 + 16, hh * 16 : hh * 16 + 16] = (
            0.2 * Wak[hh][:, None]
        )
    WS = WS.astype(np.float16)
    Wv = Wv.astype(np.float16)
    # G16[(h2,i3), i2] = (i3 == i2): gather matmul weight summing over heads
    G16 = np.zeros((128, 16), dtype=np.float16)
    for i2 in range(BLK):
        G16[i2::16, i2] = 1.0
    # HM[(h,i2),(h2,d)] = (h == h2): head block-diagonal mask
    HM = np.zeros((128, 128), dtype=np.float16)
    for hh in range(N_HEADS):
        HM[hh * 16 : (hh + 1) * 16, hh * 16 : (hh + 1) * 16] = 1.0

    in_maps = []
    for c in range(N_CORES):
        b = c // 2
        i0 = (c % 2) * HALF
        in_maps.append(
            {
                "hT": hT_all[b],
                "hTg": np.ascontiguousarray(hT_all[b][:, i0 : i0 + HALF]),
                "WlT": WlT,
                "WrT": WrT,
                "WS": WS,
                "Wv": Wv,
                "G16": G16,
                "HM": HM,
            }
        )
    return in_maps


def run_on_cores(in_maps, trace=False):
    from concourse.bass_utils import run_bass_kernel_spmd

    nc = get_program()
    return run_bass_kernel_spmd(
        nc, in_maps, core_ids=list(range(N_CORES)), trace=trace
    )


def kernel(h, mask, Wl, Wr, Wak):
    """Full-input / full-output entry point (mask is all-False by problem
    construction; masked-off attention is a no-op and is not computed)."""
    in_maps = host_prep(h, Wl, Wr, Wak)
    res = run_on_cores(in_maps, trace=False)
    full = np.empty((B, N, INPUT_DIM), dtype=np.float32)
    for c in range(N_CORES):
        b = c // 2
        i0 = (c % 2) * HALF
        full[b, i0 : i0 + HALF] = np.asarray(
            res.results[c]["out_d"], dtype=np.float32
        )
    return full
